# revision 10
# baseline (speedup 1.0000x reference)
"""Trainium2 Bass kernel for nn_Model_39676907882504.

Math: qk = (q @ k^T)/8 has shape [1,2048,1,1]; after the transposes it is
[2048,1,1,1], and softmax over the trailing size-1 axis is exactly 1.0
regardless of qk.  The final matmul with attn_weight == 1 reduces to
broadcasting `value` across a new leading dim:

    output[i, j, 0, :] = value[0, j, 0, :]   for all i in [0, 2048)

i.e. a 512KB -> 1GiB broadcast copy.  Pure memory-regime kernel.
Sharding: 256 output rows per core x 8 cores; value replicated in SBUF.

HW model (established by trace analysis + probe kernels this session):
  - A dynamic DMA instruction is split into PACKETS, one per index of the
    DRAM-side AP's outermost dim; packet i goes to SDMA engine 64+(i%16),
    restarting at 64 for every instruction.
  - 1-desc packets serialize at ~5.4us (completion latency exposed);
    7-desc packets stream with no per-packet bubble.
  - Each descriptor costs ~110ns fixed on top of bytes/27.2GB/s, so
    descriptors must be 10s-of-KB to run at port rate.
  - SBUF AXI port p serves partitions ≡ p (mod 16).  A packet walks G
    consecutive partitions; G must be coprime to 16 or engines alias
    ports.  G=7 works.  SBUF AP partition dim (dim 0) caps descs at 128.
  - Engine 79 (index 15) sustains only ~21.4 GB/s vs ~26.9 for the rest.

Kernel: the output is treated as 512 half-rows of 65536 floats.  A half-
row is written by one 7-descriptor packet: 7 overlapping windows of 9376
floats, stride 9360 (6*9360+9376 = 65536; the 16-float overlaps rewrite
identical bytes - harmless).  SBUF partition p in [0,119) holds window
vflat[65536*((p//7)%2) + 9360*(p%7) :][:9376], so any run of consecutive
half-rows maps to a run of consecutive partitions (start parity: 0 or 7).

Two instruction shapes, both 7 descs/packet (descs 37.5 KB):
  alpha: 16 packets = 16 half-rows, all 16 engines, uniform.
  gamma: 15 packets = 15 half-rows, engines 64-78, engine 79 idle.

Global mix 17 alpha + 16 gamma covers 512 half-rows exactly and loads
engine 79 with 17/33 ~= 0.52... engines 64-78 get 33 packets, engine 79
gets 17: 17*12.3us = 209us vs 33*9.8us = 322us - engine 79 comfortably
inside the envelope, everyone else at port rate.  The load streams via
the same 17x7 packet structure.
"""

import sys

for _p in ("/opt/trn_rl_repo",):
    if _p not in sys.path:
        sys.path.insert(0, _p)

import numpy as np

import bass_rust
import concourse.bass as bass
import concourse.mybir as mybir
from concourse.bass_utils import run_bass_kernel_spmd

S = 2048
D = 64
N_CORES = 8
ROWS_PER_CORE = S // N_CORES          # 256
ROW_FL = S * D                        # 131072 floats per output row
HALF_FL = ROW_FL // 2                 # 65536 floats per half-row
N_HALVES = ROWS_PER_CORE * 2          # 512

G = 7                                 # descs per packet (coprime to 16)
C = 9376                              # floats per descriptor (37504 B)
SIG = 9360                            # window stride inside a half-row
assert (G - 1) * SIG + C == HALF_FL
NPART = 119                           # SBUF window partitions (17*7)

TRACE = False          # test.py flips this to profile
TRACE_KWARGS = {}
LAST_RESULT = None     # BassKernelResults of the last run (for test.py)


def build_program():
    nc = bass.Bass()
    val = nc.declare_dram_parameter("value_w", [NPART, C], mybir.dt.float32,
                                    isOutput=False)
    out = nc.declare_dram_parameter("out", [N_HALVES, HALF_FL],
                                    mybir.dt.float32, isOutput=True)
    wt = nc.alloc_sbuf_tensor("wt", [NPART, C], mybir.dt.float32)

    def store(eng, h0, npk):
        """one instruction: half-rows h0..h0+npk-1 (npk packets x 7 descs)"""
        p0 = G * (h0 % 2)
        o = out[h0:h0 + npk, 0:HALF_FL]
        o.ap = bass_rust.VecI64Pair([[HALF_FL, npk], [SIG, G], [1, C]])
        return eng.dma_start(out=o, in_=wt[p0:p0 + npk * G, 0:C])

    def load(eng):
        i = val[0:NPART, 0:C]
        i.ap = bass_rust.VecI64Pair([[G * C, 17], [C, G], [1, C]])
        return eng.dma_start(out=wt[:, :], in_=i)

    def plan(h0, n_alpha, n_gamma):
        """interleaved alpha(16)/gamma(15) instruction sizes"""
        sizes = []
        a, g = n_alpha, n_gamma
        while a or g:
            if a:
                sizes.append(16); a -= 1
            if g:
                sizes.append(15); g -= 1
        starts = []
        h = h0
        for sz in sizes:
            starts.append((h, sz)); h += sz
        return starts, h

    q1, h_mid = plan(0, 8, 8)                  # 248 half-rows
    q2, h_end = plan(h_mid, 9, 8)              # 264 half-rows
    assert h_end == N_HALVES

    with nc.Block() as block, nc.semaphore("dma_sem") as dma_sem, \
            nc.semaphore("scr_sem") as scr_sem:

        @block.sync
        def _(sync):
            load(sync).then_inc(dma_sem, 16)
            sync.wait_ge(dma_sem, 16)
            for h0, npk in q1:
                store(sync, h0, npk).then_inc(scr_sem, 16)
            store(sync, 0, 16).then_inc(dma_sem, 16)      # closer (rewrite)
            sync.wait_ge(dma_sem, 48)

        @block.scalar
        def _(scalar):
            scalar.wait_ge(dma_sem, 16)
            for h0, npk in q2:
                store(scalar, h0, npk).then_inc(scr_sem, 16)
            store(scalar, 16, 16).then_inc(dma_sem, 16)   # closer (rewrite)
            scalar.wait_ge(dma_sem, 48)

    return nc


def _pack_value(value):
    vflat = np.ascontiguousarray(np.asarray(value, np.float32)).reshape(ROW_FL)
    w = np.zeros((NPART, C), np.float32)
    for p in range(NPART):
        off = HALF_FL * ((p // G) % 2) + SIG * (p % G)
        w[p] = vflat[off: off + C]
    return w


def kernel(query=None, key=None, value=None, attn_mask=None, **_ignored):
    global LAST_RESULT
    vw = _pack_value(value)

    nc = build_program()
    core_ids = list(range(N_CORES))
    in_maps = [{"value_w": vw} for _ in core_ids]
    res = run_bass_kernel_spmd(nc, in_maps, core_ids, trace=TRACE,
                               **TRACE_KWARGS)
    LAST_RESULT = res

    shards = [res.results[i]["out"].reshape(ROWS_PER_CORE, S, 1, D)
              for i in range(N_CORES)]
    return np.concatenate(shards, axis=0)
